# revision 3
# baseline (speedup 1.0000x reference)
"""Expert-parallel MoE FFN kernel for 8 Trainium2 NeuronCores.

Problem: x (B=4, E=8, N=1024, D=1024) f32; per-expert 2-layer GELU FFN
  h = gelu(x[:,e] @ w1[e] + b1[e]);  out[:,e] = h @ w2[e] + b2[e]
with w1 (E, D, H=4096), w2 (E, H, D).

Sharding: expert-parallel, one expert per core (E == n_cores == 8). Each
core's work is fully independent — no collectives.

Per-core device kernel (fused, weights resident in SBUF, bf16 matmuls with
fp32 PSUM accumulation):
  - host sends xT_e = x_e.T (D, NTOK) bf16 so no on-device transposes needed
  - mm1: hT (H, TN-chunk) = w1.T @ xT   [lhsT = w1 block, rhs = xT chunk]
  - gelu(tanh approx) + b1 on ScalarE during PSUM->SBUF eviction (bf16)
  - mm2: out (TN-chunk, D) = hT.T @ w2  [lhsT = hT block, rhs = w2 block]
  - + b2 on VectorE during PSUM->SBUF eviction (f32), DMA to DRAM
"""

import numpy as np
import ml_dtypes

B, E, N, D, H = 4, 8, 1024, 1024, 4096
NTOK = B * N            # 4096 tokens per expert
P = 128
TN = 256                # token chunk = mm1 moving free dim
NCH = NTOK // TN        # 16 chunks
KO1 = D // P            # 8 k-subtiles for mm1 (contract over D)
KO2 = H // P            # 32 k-subtiles for mm2 (contract over H) = mm1 m-tiles
MT = TN // P            # 2 token subtiles per chunk
NF2 = 512               # mm2 moving free dim (over D)
ND = D // NF2           # 2

_CACHE: dict = {}


def _build_nc():
    import concourse.mybir as mybir
    import concourse.tile as tile
    from concourse import bacc

    bf16 = mybir.dt.bfloat16
    f32 = mybir.dt.float32
    gelu = mybir.ActivationFunctionType.Gelu_apprx_tanh
    alu_add = mybir.AluOpType.add

    nc = bacc.Bacc(None, target_bir_lowering=False, debug=False)

    xT = nc.dram_tensor("xT", [D, NTOK], bf16, kind="ExternalInput")
    w1 = nc.dram_tensor("w1", [D, H], bf16, kind="ExternalInput")
    b1 = nc.dram_tensor("b1", [P, KO2], f32, kind="ExternalInput")
    w2 = nc.dram_tensor("w2", [H, D], bf16, kind="ExternalInput")
    b2 = nc.dram_tensor("b2", [P, D], f32, kind="ExternalInput")
    out = nc.dram_tensor("out", [NTOK, D], f32, kind="ExternalOutput")

    xT_v = xT.rearrange("(ko pi) n -> pi ko n", pi=P)     # (128, 8, 4096)
    w1_v = w1.rearrange("(ko pi) h -> pi ko h", pi=P)     # (128, 8, 4096)
    w2_v = w2.rearrange("(ko pi) d -> pi ko d", pi=P)     # (128, 32, 1024)
    out_v = out.rearrange("(mt pi) d -> pi mt d", pi=P)   # (128, 32, 1024)

    with tile.TileContext(nc) as tc:
        with (
            tc.tile_pool(name="wpool", bufs=1) as wpool,
            tc.tile_pool(name="xpool", bufs=2) as xpool,
            tc.tile_pool(name="hpool", bufs=4) as hpool,
            tc.tile_pool(name="opool", bufs=2) as opool,
            tc.tile_pool(name="phpool", bufs=3, space="PSUM") as phpool,
            tc.tile_pool(name="popool", bufs=1, space="PSUM") as popool,
        ):
            w1_sb = wpool.tile([P, KO1, H], bf16, name="w1_sb")
            w2_sb = wpool.tile([P, KO2, D], bf16, name="w2_sb")
            b1_sb = wpool.tile([P, KO2], f32, name="b1_sb")
            b2_sb = wpool.tile([P, D], f32, name="b2_sb")
            # split weight loads so they spread across DMA queues
            for ko in range(KO1):
                nc.sync.dma_start(out=w1_sb[:, ko, :], in_=w1_v[:, ko, :])
            for ko in range(KO2):
                nc.sync.dma_start(out=w2_sb[:, ko, :], in_=w2_v[:, ko, :])
            nc.sync.dma_start(out=b1_sb[:], in_=b1[:])
            nc.sync.dma_start(out=b2_sb[:], in_=b2[:])

            for t in range(NCH):
                x_sb = xpool.tile([P, KO1, TN], bf16, tag="x", name=f"x_sb{t}")
                nc.sync.dma_start(
                    out=x_sb[:], in_=xT_v[:, :, t * TN : (t + 1) * TN]
                )
                po = [
                    popool.tile([P, D], f32, tag=f"po{mt}", name=f"po{mt}_{t}")
                    for mt in range(MT)
                ]
                for m in range(KO2):
                    ph = phpool.tile([P, TN], f32, tag="ph", name=f"ph_{t}_{m}")
                    for ko in range(KO1):
                        nc.tensor.matmul(
                            ph[:],
                            w1_sb[:, ko, m * P : (m + 1) * P],
                            x_sb[:, ko, :],
                            start=(ko == 0),
                            stop=(ko == KO1 - 1),
                        )
                    h_sb = hpool.tile([P, TN], bf16, tag="h", name=f"h_{t}_{m}")
                    nc.scalar.activation(
                        h_sb[:], ph[:], gelu, bias=b1_sb[:, m : m + 1], scale=1.0
                    )
                    for mt in range(MT):
                        for n in range(ND):
                            nc.tensor.matmul(
                                po[mt][:, n * NF2 : (n + 1) * NF2],
                                h_sb[:, mt * P : (mt + 1) * P],
                                w2_sb[:, m, n * NF2 : (n + 1) * NF2],
                                start=(m == 0),
                                stop=(m == KO2 - 1),
                            )
                for mt in range(MT):
                    o_sb = opool.tile([P, D], f32, tag=f"o{mt}", name=f"o{mt}_{t}")
                    nc.vector.tensor_tensor(o_sb[:], po[mt][:], b2_sb[:], alu_add)
                    nc.sync.dma_start(
                        out=out_v[:, t * MT + mt, :], in_=o_sb[:]
                    )

    nc.compile()
    return nc


def _get_nc():
    if "nc" not in _CACHE:
        _CACHE["nc"] = _build_nc()
    return _CACHE["nc"]


def _get_runner():
    """Build (once) a cached jitted SPMD executor for the Bass module.

    Mirrors concourse.bass2jax.run_bass_via_pjrt's multi-core branch, but
    caches the jitted function so repeat calls don't re-trace/re-compile.
    """
    if "runner" in _CACHE:
        return _CACHE["runner"]
    import jax
    from jax.experimental.shard_map import shard_map
    from jax.sharding import Mesh, NamedSharding, PartitionSpec

    import concourse.mybir as mybir
    from concourse import bass2jax

    bass2jax.install_neuronx_cc_hook()
    nc = _get_nc()

    partition_name = (
        nc.partition_id_tensor.name if nc.partition_id_tensor else None
    )
    in_names = []
    out_names = []
    out_avals = []
    zero_out_specs = []
    for alloc in nc.m.functions[0].allocations:
        if not isinstance(alloc, mybir.MemoryLocationSet):
            continue
        name = alloc.memorylocations[0].name
        if alloc.kind == "ExternalInput":
            if name != partition_name:
                in_names.append(name)
        elif alloc.kind == "ExternalOutput":
            shape = tuple(alloc.tensor_shape)
            dtype = mybir.dt.np(alloc.dtype)
            out_names.append(name)
            out_avals.append(jax.core.ShapedArray(shape, dtype))
            zero_out_specs.append((shape, dtype))
    n_params = len(in_names)
    n_outs = len(out_names)
    all_in_names = list(in_names) + list(out_names)
    if partition_name is not None:
        all_in_names.append(partition_name)
    donate = tuple(range(n_params, n_params + n_outs))

    def _body(*args):
        operands = list(args)
        if partition_name is not None:
            operands.append(bass2jax.partition_id_tensor())
        outs = bass2jax._bass_exec_p.bind(
            *operands,
            out_avals=tuple(out_avals),
            in_names=tuple(all_in_names),
            out_names=tuple(out_names),
            lowering_input_output_aliases=(),
            sim_require_finite=True,
            sim_require_nnan=True,
            nc=nc,
        )
        return tuple(outs)

    devices = jax.devices()[:E]
    mesh = Mesh(np.asarray(devices), ("core",))
    in_specs = (PartitionSpec("core"),) * (n_params + n_outs)
    out_specs = (PartitionSpec("core"),) * n_outs
    fn = jax.jit(
        shard_map(
            _body, mesh=mesh, in_specs=in_specs, out_specs=out_specs,
            check_rep=False,
        ),
        donate_argnums=donate,
        keep_unused=True,
    )
    sharding = NamedSharding(mesh, PartitionSpec("core"))
    runner = {
        "fn": fn,
        "in_names": in_names,
        "out_names": out_names,
        "out_avals": out_avals,
        "zero_out_specs": zero_out_specs,
        "sharding": sharding,
    }
    _CACHE["runner"] = runner
    return runner


def _exec_spmd(in_maps):
    """Run the cached executor on per-core input dicts; returns per-core
    output dicts."""
    import jax

    r = _get_runner()
    concat_in = [
        np.concatenate([np.asarray(m[name]) for m in in_maps], axis=0)
        for name in r["in_names"]
    ]
    zeros = [
        np.zeros((E * shape[0], *shape[1:]), dtype)
        for shape, dtype in r["zero_out_specs"]
    ]
    out_arrs = r["fn"](*concat_in, *zeros)
    results = []
    for c in range(E):
        results.append(
            {
                name: np.asarray(out_arrs[i]).reshape(
                    E, *r["out_avals"][i].shape
                )[c]
                for i, name in enumerate(r["out_names"])
            }
        )
    return results


def _prepare_in_maps(x, w1, b1, w2, b2):
    bf16 = ml_dtypes.bfloat16
    in_maps = []
    for e in range(E):
        x_e = np.ascontiguousarray(x[:, e].reshape(NTOK, D).T).astype(bf16)
        w1_e = np.ascontiguousarray(w1[e]).astype(bf16)
        w2_e = np.ascontiguousarray(w2[e]).astype(bf16)
        b1_e = np.ascontiguousarray(
            b1[e].astype(np.float32).reshape(KO2, P).T
        )
        b2_e = np.ascontiguousarray(
            np.broadcast_to(b2[e].astype(np.float32), (P, D))
        )
        in_maps.append(
            {"xT": x_e, "w1": w1_e, "b1": b1_e, "w2": w2_e, "b2": b2_e}
        )
    return in_maps


def _run(x, w1, b1, w2, b2):
    in_maps = _prepare_in_maps(x, w1, b1, w2, b2)
    results = _exec_spmd(in_maps)
    out = np.empty((B, E, N, D), dtype=np.float32)
    for e in range(E):
        out[:, e] = results[e]["out"].reshape(B, N, D)
    return out


def kernel(x, w1, b1, w2, b2):
    return _run(x, w1, b1, w2, b2)


# revision 10
# speedup vs baseline: 1.1003x; 1.1003x over previous
"""Expert-parallel MoE FFN kernel for 8 Trainium2 NeuronCores.

Problem: x (B=4, E=8, N=1024, D=1024) f32; per-expert 2-layer GELU FFN
  h = gelu(x[:,e] @ w1[e] + b1[e]);  out[:,e] = h @ w2[e] + b2[e]
with w1 (E, D, H=4096), w2 (E, H, D).

Sharding: expert-parallel, one expert per core (E == n_cores == 8). Each
core's work is fully independent — no collectives.

Per-core device kernel (fused, weights resident in SBUF, bf16 matmuls with
fp32 PSUM accumulation):
  - host sends xT_e = x_e.T (D, NTOK) bf16 so no on-device transposes needed
  - mm1: hT (H, TN-chunk) = w1.T @ xT   [lhsT = w1 block, rhs = xT chunk]
  - gelu(tanh approx) + b1 on ScalarE during PSUM->SBUF eviction (bf16)
  - mm2: out (TN-chunk, D) = hT.T @ w2  [lhsT = hT block, rhs = w2 block]
  - + b2 on VectorE during PSUM->SBUF eviction (f32), DMA to DRAM
"""

import numpy as np
import ml_dtypes

B, E, N, D, H = 4, 8, 1024, 1024, 4096
NTOK = B * N            # 4096 tokens per expert
P = 128
TN = 256                # token chunk = mm1 moving free dim
NCH = NTOK // TN        # 16 chunks
KO1 = D // P            # 8 k-subtiles for mm1 (contract over D)
KO2 = H // P            # 32 k-subtiles for mm2 (contract over H) = mm1 m-tiles
MT = TN // P            # 2 token subtiles per chunk
NF2 = 512               # mm2 moving free dim (over D)
ND = D // NF2           # 2

_CACHE: dict = {}


def _build_nc(reps=1):
    """Build the per-core Bass program. reps>1 repeats the (idempotent)
    kernel body for marginal-time benchmarking."""
    import concourse.mybir as mybir
    import concourse.tile as tile
    from concourse import bacc

    bf16 = mybir.dt.bfloat16
    f32 = mybir.dt.float32
    gelu = mybir.ActivationFunctionType.Gelu_apprx_tanh
    alu_add = mybir.AluOpType.add

    nc = bacc.Bacc(None, target_bir_lowering=False, debug=False)

    xT = nc.dram_tensor("xT", [D, NTOK], bf16, kind="ExternalInput")
    w1 = nc.dram_tensor("w1", [D, H], bf16, kind="ExternalInput")
    b1 = nc.dram_tensor("b1", [P, KO2], f32, kind="ExternalInput")
    w2 = nc.dram_tensor("w2", [H, D], bf16, kind="ExternalInput")
    b2 = nc.dram_tensor("b2", [P, D], f32, kind="ExternalInput")
    out = nc.dram_tensor("out", [NTOK, D], f32, kind="ExternalOutput")

    xT_v = xT.rearrange("(ko pi) n -> pi ko n", pi=P)     # (128, 8, 4096)
    w1_v = w1.rearrange("(ko pi) h -> pi ko h", pi=P)     # (128, 8, 4096)
    w2_v = w2.rearrange("(ko pi) d -> pi ko d", pi=P)     # (128, 32, 1024)
    out_v = out.rearrange("(mt pi) d -> pi mt d", pi=P)   # (128, 32, 1024)

    with tile.TileContext(nc) as tc:
        with (
            tc.tile_pool(name="wpool", bufs=1) as wpool,
            tc.tile_pool(name="xpool", bufs=2) as xpool,
            tc.tile_pool(name="hpool", bufs=6) as hpool,
            tc.tile_pool(name="opool", bufs=2) as opool,
            tc.tile_pool(name="phpool", bufs=2, space="PSUM") as phpool,
            tc.tile_pool(name="popool", bufs=1, space="PSUM") as popool,
            tc.tile_pool(name="popool2", bufs=2, space="PSUM") as popool2,
        ):
            # per-ko weight tiles -> fine-grained DMA deps (PE can start as
            # soon as the pieces it needs have landed, not the whole matrix)
            w1_sb = [wpool.tile([P, H], bf16, name=f"w1_sb{ko}") for ko in range(KO1)]
            w2_sb = [wpool.tile([P, D], bf16, name=f"w2_sb{ko}") for ko in range(KO2)]
            b1_sb = wpool.tile([P, KO2], f32, name="b1_sb")
            b2_sb = wpool.tile([P, D], f32, name="b2_sb")

            def load_x(rep, t):
                xs = [
                    xpool.tile([P, TN], bf16, tag=f"x{ko}", name=f"x_{rep}_{t}_{ko}")
                    for ko in range(KO1)
                ]
                for ko in range(KO1):
                    nc.sync.dma_start(
                        out=xs[ko][:], in_=xT_v[:, ko, t * TN : (t + 1) * TN]
                    )
                return xs

            # first x chunk + w1 pieces first: mm1(t=0) can start ~ASAP
            x_next = load_x(0, 0)
            for ko in range(KO1):
                nc.sync.dma_start(out=w1_sb[ko][:], in_=w1_v[:, ko, :])
            nc.sync.dma_start(out=b1_sb[:], in_=b1[:])
            for ko in range(KO2):
                nc.sync.dma_start(out=w2_sb[ko][:], in_=w2_v[:, ko, :])
            nc.sync.dma_start(out=b2_sb[:], in_=b2[:])

            # software pipeline: mm2 lags mm1 by LAG m-steps (across chunk
            # boundaries too) so PE never waits on the ScalarE gelu evict or
            # the previous chunk's PSUM eviction.
            LAG = 2
            pend_q = []  # entries: (h_sb, po, m, rep, t)

            def emit_mm2(h_sb, po, m, rep, t, final):
                for mt in range(MT):
                    for n in range(ND):
                        nc.tensor.matmul(
                            po[mt][:, n * NF2 : (n + 1) * NF2],
                            h_sb[:, mt * P : (mt + 1) * P],
                            w2_sb[m][:, n * NF2 : (n + 1) * NF2],
                            start=(m == 0),
                            stop=(m == KO2 - 1),
                        )
                    if final:
                        # evict this mt's accumulator right away (frees its
                        # PSUM slot before the next chunk's mm2 needs it)
                        o_sb = opool.tile(
                            [P, D], f32, tag=f"o{mt}", name=f"o{mt}_{rep}_{t}"
                        )
                        nc.vector.tensor_tensor(
                            o_sb[:], po[mt][:], b2_sb[:], alu_add
                        )
                        nc.sync.dma_start(
                            out=out_v[:, t * MT + mt, :], in_=o_sb[:]
                        )

            def pump(force=False):
                while pend_q and (force or len(pend_q) > LAG):
                    h_sb, po, m, rep, t = pend_q.pop(0)
                    emit_mm2(h_sb, po, m, rep, t, final=(m == KO2 - 1))

            for rep in range(reps):
              for t in range(NCH):
                x_sb = x_next
                po = [
                    popool.tile([P, D], f32, tag="po0", name=f"po0_{rep}_{t}"),
                    popool2.tile([P, D], f32, tag="po1", name=f"po1_{rep}_{t}"),
                ]
                for m in range(KO2):
                    ph = phpool.tile([P, TN], f32, tag="ph", name=f"ph_{rep}_{t}_{m}")
                    for ko in range(KO1):
                        nc.tensor.matmul(
                            ph[:],
                            w1_sb[ko][:, m * P : (m + 1) * P],
                            x_sb[ko][:],
                            start=(ko == 0),
                            stop=(ko == KO1 - 1),
                        )
                    h_sb = hpool.tile([P, TN], bf16, tag="h", name=f"h_{rep}_{t}_{m}")
                    nc.scalar.activation(
                        h_sb[:], ph[:], gelu, bias=b1_sb[:, m : m + 1], scale=1.0
                    )
                    pend_q.append((h_sb, po, m, rep, t))
                    pump()
                    if m == 0 and not (t == NCH - 1 and rep == reps - 1):
                        # prefetch next chunk's x while this chunk computes
                        tn, rn = (t + 1, rep) if t < NCH - 1 else (0, rep + 1)
                        x_next = load_x(rn, tn)
            pump(force=True)

    nc.compile()
    return nc


def _get_nc(reps=1):
    key = f"nc{reps}"
    if key not in _CACHE:
        _CACHE[key] = _build_nc(reps)
    return _CACHE[key]


def _get_runner(reps=1):
    """Build (once) a cached jitted SPMD executor for the Bass module.

    Mirrors concourse.bass2jax.run_bass_via_pjrt's multi-core branch, but
    caches the jitted function so repeat calls don't re-trace/re-compile.
    """
    key = f"runner{reps}"
    if key in _CACHE:
        return _CACHE[key]
    import jax
    from jax.experimental.shard_map import shard_map
    from jax.sharding import Mesh, NamedSharding, PartitionSpec

    import concourse.mybir as mybir
    from concourse import bass2jax

    bass2jax.install_neuronx_cc_hook()
    nc = _get_nc(reps)

    partition_name = (
        nc.partition_id_tensor.name if nc.partition_id_tensor else None
    )
    in_names = []
    out_names = []
    out_avals = []
    zero_out_specs = []
    for alloc in nc.m.functions[0].allocations:
        if not isinstance(alloc, mybir.MemoryLocationSet):
            continue
        name = alloc.memorylocations[0].name
        if alloc.kind == "ExternalInput":
            if name != partition_name:
                in_names.append(name)
        elif alloc.kind == "ExternalOutput":
            shape = tuple(alloc.tensor_shape)
            dtype = mybir.dt.np(alloc.dtype)
            out_names.append(name)
            out_avals.append(jax.core.ShapedArray(shape, dtype))
            zero_out_specs.append((shape, dtype))
    n_params = len(in_names)
    n_outs = len(out_names)
    all_in_names = list(in_names) + list(out_names)
    if partition_name is not None:
        all_in_names.append(partition_name)
    donate = tuple(range(n_params, n_params + n_outs))

    def _body(*args):
        operands = list(args)
        if partition_name is not None:
            operands.append(bass2jax.partition_id_tensor())
        outs = bass2jax._bass_exec_p.bind(
            *operands,
            out_avals=tuple(out_avals),
            in_names=tuple(all_in_names),
            out_names=tuple(out_names),
            lowering_input_output_aliases=(),
            sim_require_finite=True,
            sim_require_nnan=True,
            nc=nc,
        )
        return tuple(outs)

    devices = jax.devices()[:E]
    mesh = Mesh(np.asarray(devices), ("core",))
    in_specs = (PartitionSpec("core"),) * (n_params + n_outs)
    out_specs = (PartitionSpec("core"),) * n_outs
    fn = jax.jit(
        shard_map(
            _body, mesh=mesh, in_specs=in_specs, out_specs=out_specs,
            check_rep=False,
        ),
        donate_argnums=donate,
        keep_unused=True,
    )
    sharding = NamedSharding(mesh, PartitionSpec("core"))
    runner = {
        "fn": fn,
        "in_names": in_names,
        "out_names": out_names,
        "out_avals": out_avals,
        "zero_out_specs": zero_out_specs,
        "sharding": sharding,
    }
    _CACHE[key] = runner
    return runner


def _exec_spmd(in_maps, reps=1):
    """Run the cached executor on per-core input dicts; returns per-core
    output dicts."""
    import jax

    r = _get_runner(reps)
    concat_in = [
        np.concatenate([np.asarray(m[name]) for m in in_maps], axis=0)
        for name in r["in_names"]
    ]
    zeros = [
        np.zeros((E * shape[0], *shape[1:]), dtype)
        for shape, dtype in r["zero_out_specs"]
    ]
    out_arrs = r["fn"](*concat_in, *zeros)
    results = []
    for c in range(E):
        results.append(
            {
                name: np.asarray(out_arrs[i]).reshape(
                    E, *r["out_avals"][i].shape
                )[c]
                for i, name in enumerate(r["out_names"])
            }
        )
    return results


def _prepare_in_maps(x, w1, b1, w2, b2):
    bf16 = ml_dtypes.bfloat16
    in_maps = []
    for e in range(E):
        x_e = np.ascontiguousarray(x[:, e].reshape(NTOK, D).T).astype(bf16)
        w1_e = np.ascontiguousarray(w1[e]).astype(bf16)
        w2_e = np.ascontiguousarray(w2[e]).astype(bf16)
        b1_e = np.ascontiguousarray(
            b1[e].astype(np.float32).reshape(KO2, P).T
        )
        b2_e = np.ascontiguousarray(
            np.broadcast_to(b2[e].astype(np.float32), (P, D))
        )
        in_maps.append(
            {"xT": x_e, "w1": w1_e, "b1": b1_e, "w2": w2_e, "b2": b2_e}
        )
    return in_maps


def _run(x, w1, b1, w2, b2):
    in_maps = _prepare_in_maps(x, w1, b1, w2, b2)
    results = _exec_spmd(in_maps)
    out = np.empty((B, E, N, D), dtype=np.float32)
    for e in range(E):
        out[:, e] = results[e]["out"].reshape(B, N, D)
    return out


def kernel(x, w1, b1, w2, b2):
    return _run(x, w1, b1, w2, b2)


# revision 16
# speedup vs baseline: 120.4296x; 109.4536x over previous
"""Expert-parallel MoE FFN kernel for 8 Trainium2 NeuronCores.

Problem: x (B=4, E=8, N=1024, D=1024) f32; per-expert 2-layer GELU FFN
  h = gelu(x[:,e] @ w1[e] + b1[e]);  out[:,e] = h @ w2[e] + b2[e]
with w1 (E, D, H=4096), w2 (E, H, D).

Sharding: expert-parallel, one expert per core (E == n_cores == 8). Each
core's work is fully independent — no collectives.

Per-core device kernel (fused, weights resident in SBUF, bf16 matmuls with
fp32 PSUM accumulation):
  - host sends xT_e = x_e.T (D, NTOK) bf16 so no on-device transposes needed
  - mm1: hT (H, TN-chunk) = w1.T @ xT   [lhsT = w1 block, rhs = xT chunk]
  - gelu(tanh approx) + b1 on ScalarE during PSUM->SBUF eviction (bf16)
  - mm2: out (TN-chunk, D) = hT.T @ w2  [lhsT = hT block, rhs = w2 block]
  - + b2 on VectorE during PSUM->SBUF eviction (f32), DMA to DRAM
"""

import numpy as np
import ml_dtypes

B, E, N, D, H = 4, 8, 1024, 1024, 4096
NTOK = B * N            # 4096 tokens per expert
P = 128
TN = 256                # token chunk = mm1 moving free dim
NCH = NTOK // TN        # 16 chunks
KO1 = D // P            # 8 k-subtiles for mm1 (contract over D)
KO2 = H // P            # 32 k-subtiles for mm2 (contract over H) = mm1 m-tiles
MT = TN // P            # 2 token subtiles per chunk
NF2 = 512               # mm2 moving free dim (over D)
ND = D // NF2           # 2

_CACHE: dict = {}


def _build_nc(reps=1):
    """Build the per-core Bass program. reps>1 repeats the (idempotent)
    kernel body for marginal-time benchmarking."""
    import concourse.mybir as mybir
    import concourse.tile as tile
    from concourse import bacc

    bf16 = mybir.dt.bfloat16
    f32 = mybir.dt.float32
    gelu = mybir.ActivationFunctionType.Gelu_apprx_tanh
    alu_add = mybir.AluOpType.add

    nc = bacc.Bacc(None, target_bir_lowering=False, debug=False)

    xT = nc.dram_tensor("xT", [D, NTOK], bf16, kind="ExternalInput")
    w1 = nc.dram_tensor("w1", [D, H], bf16, kind="ExternalInput")
    b1 = nc.dram_tensor("b1", [P, KO2], f32, kind="ExternalInput")
    w2 = nc.dram_tensor("w2", [H, D], bf16, kind="ExternalInput")
    b2 = nc.dram_tensor("b2", [P, D], f32, kind="ExternalInput")
    out = nc.dram_tensor("out", [NTOK, D], f32, kind="ExternalOutput")

    xT_v = xT.rearrange("(ko pi) n -> pi ko n", pi=P)     # (128, 8, 4096)
    w1_v = w1.rearrange("(ko pi) h -> pi ko h", pi=P)     # (128, 8, 4096)
    w2_v = w2.rearrange("(ko pi) d -> pi ko d", pi=P)     # (128, 32, 1024)
    out_v = out.rearrange("(mt pi) d -> pi mt d", pi=P)   # (128, 32, 1024)

    with tile.TileContext(nc) as tc:
        with (
            tc.tile_pool(name="wpool", bufs=1) as wpool,
            tc.tile_pool(name="xpool", bufs=2) as xpool,
            tc.tile_pool(name="hpool", bufs=6) as hpool,
            tc.tile_pool(name="opool", bufs=2) as opool,
            tc.tile_pool(name="phpool", bufs=2, space="PSUM") as phpool,
            tc.tile_pool(name="popool", bufs=1, space="PSUM") as popool,
            tc.tile_pool(name="popool2", bufs=2, space="PSUM") as popool2,
        ):
            # w1 split [ko][mg]: per-ko tiles chopped into H-column groups so
            # chunk 0's first matmuls only wait for the first ~1MB, not all
            # 8.4MB of w1. w2 split per-ko (mm2(m) waits only on piece m).
            MG = 4           # m-groups for w1 (H columns per group = H/MG)
            HG = H // MG
            w1_sb = [
                [wpool.tile([P, HG], bf16, name=f"w1_sb{ko}_{mg}") for mg in range(MG)]
                for ko in range(KO1)
            ]
            w2_sb = [wpool.tile([P, D], bf16, name=f"w2_sb{ko}") for ko in range(KO2)]
            b1_sb = wpool.tile([P, KO2], f32, name="b1_sb")
            b2_sb = wpool.tile([P, D], f32, name="b2_sb")

            def load_x(rep, t):
                xs = [
                    xpool.tile([P, TN], bf16, tag=f"x{ko}", name=f"x_{rep}_{t}_{ko}")
                    for ko in range(KO1)
                ]
                for ko in range(KO1):
                    nc.sync.dma_start(
                        out=xs[ko][:], in_=xT_v[:, ko, t * TN : (t + 1) * TN]
                    )
                return xs

            # DMA issue order = consumption order: x chunk 0, b1, then w1
            # m-group by m-group, with w2 pieces interleaved after the first
            # w1 group (mm2(m) starts ~2 m-steps after mm1(m)).
            x_next = load_x(0, 0)
            nc.sync.dma_start(out=b1_sb[:], in_=b1[:])
            for ko in range(KO1):
                nc.sync.dma_start(
                    out=w1_sb[ko][0][:], in_=w1_v[:, ko, 0:HG]
                )
            nc.sync.dma_start(out=b2_sb[:], in_=b2[:])
            for mg in range(1, MG):
                for ko in range(KO1):
                    nc.sync.dma_start(
                        out=w1_sb[ko][mg][:], in_=w1_v[:, ko, mg * HG : (mg + 1) * HG]
                    )
                # interleave a share of w2 pieces after each w1 group
                for ko in range((mg - 1) * KO2 // (MG - 1), mg * KO2 // (MG - 1)):
                    nc.sync.dma_start(out=w2_sb[ko][:], in_=w2_v[:, ko, :])

            # software pipeline: mm2 lags mm1 by LAG m-steps (across chunk
            # boundaries too) so PE never waits on the ScalarE gelu evict or
            # the previous chunk's PSUM eviction.
            LAG = 2
            pend_q = []  # entries: (h_sb, po, m, rep, t)

            def emit_mm2(h_sb, po, m, rep, t, final):
                for mt in range(MT):
                    for n in range(ND):
                        nc.tensor.matmul(
                            po[mt][:, n * NF2 : (n + 1) * NF2],
                            h_sb[:, mt * P : (mt + 1) * P],
                            w2_sb[m][:, n * NF2 : (n + 1) * NF2],
                            start=(m == 0),
                            stop=(m == KO2 - 1),
                        )
                    if final:
                        # evict this mt's accumulator right away (frees its
                        # PSUM slot before the next chunk's mm2 needs it)
                        o_sb = opool.tile(
                            [P, D], f32, tag=f"o{mt}", name=f"o{mt}_{rep}_{t}"
                        )
                        nc.vector.tensor_tensor(
                            o_sb[:], po[mt][:], b2_sb[:], alu_add
                        )
                        nc.sync.dma_start(
                            out=out_v[:, t * MT + mt, :], in_=o_sb[:]
                        )

            def pump(force=False):
                while pend_q and (force or len(pend_q) > LAG):
                    h_sb, po, m, rep, t = pend_q.pop(0)
                    emit_mm2(h_sb, po, m, rep, t, final=(m == KO2 - 1))

            for rep in range(reps):
              for t in range(NCH):
                x_sb = x_next
                po = [
                    popool.tile([P, D], f32, tag="po0", name=f"po0_{rep}_{t}"),
                    popool2.tile([P, D], f32, tag="po1", name=f"po1_{rep}_{t}"),
                ]
                for m in range(KO2):
                    mg, mo = divmod(m, KO2 // MG)
                    ph = phpool.tile([P, TN], f32, tag="ph", name=f"ph_{rep}_{t}_{m}")
                    for ko in range(KO1):
                        nc.tensor.matmul(
                            ph[:],
                            w1_sb[ko][mg][:, mo * P : (mo + 1) * P],
                            x_sb[ko][:],
                            start=(ko == 0),
                            stop=(ko == KO1 - 1),
                        )
                    h_sb = hpool.tile([P, TN], bf16, tag="h", name=f"h_{rep}_{t}_{m}")
                    nc.scalar.activation(
                        h_sb[:], ph[:], gelu, bias=b1_sb[:, m : m + 1], scale=1.0
                    )
                    pend_q.append((h_sb, po, m, rep, t))
                    pump()
                    if m == 0 and not (t == NCH - 1 and rep == reps - 1):
                        # prefetch next chunk's x while this chunk computes
                        tn, rn = (t + 1, rep) if t < NCH - 1 else (0, rep + 1)
                        x_next = load_x(rn, tn)
            pump(force=True)

    nc.compile()
    return nc


def _get_nc(reps=1):
    key = f"nc{reps}"
    if key not in _CACHE:
        _CACHE[key] = _build_nc(reps)
    return _CACHE[key]


def _make_runner(nc):
    """Build a jitted SPMD executor for an arbitrary finalized Bass module.

    Mirrors concourse.bass2jax.run_bass_via_pjrt's multi-core branch, but
    returns a reusable jitted function (no re-trace/re-compile per call).
    """
    import jax
    from jax.experimental.shard_map import shard_map
    from jax.sharding import Mesh, NamedSharding, PartitionSpec

    import concourse.mybir as mybir
    from concourse import bass2jax

    bass2jax.install_neuronx_cc_hook()

    partition_name = (
        nc.partition_id_tensor.name if nc.partition_id_tensor else None
    )
    in_names = []
    out_names = []
    out_avals = []
    zero_out_specs = []
    for alloc in nc.m.functions[0].allocations:
        if not isinstance(alloc, mybir.MemoryLocationSet):
            continue
        name = alloc.memorylocations[0].name
        if alloc.kind == "ExternalInput":
            if name != partition_name:
                in_names.append(name)
        elif alloc.kind == "ExternalOutput":
            shape = tuple(alloc.tensor_shape)
            dtype = mybir.dt.np(alloc.dtype)
            out_names.append(name)
            out_avals.append(jax.core.ShapedArray(shape, dtype))
            zero_out_specs.append((shape, dtype))
    n_params = len(in_names)
    n_outs = len(out_names)
    all_in_names = list(in_names) + list(out_names)
    if partition_name is not None:
        all_in_names.append(partition_name)
    donate = tuple(range(n_params, n_params + n_outs))

    def _body(*args):
        operands = list(args)
        if partition_name is not None:
            operands.append(bass2jax.partition_id_tensor())
        outs = bass2jax._bass_exec_p.bind(
            *operands,
            out_avals=tuple(out_avals),
            in_names=tuple(all_in_names),
            out_names=tuple(out_names),
            lowering_input_output_aliases=(),
            sim_require_finite=True,
            sim_require_nnan=True,
            nc=nc,
        )
        return tuple(outs)

    devices = jax.devices()[:E]
    mesh = Mesh(np.asarray(devices), ("core",))
    in_specs = (PartitionSpec("core"),) * (n_params + n_outs)
    out_specs = (PartitionSpec("core"),) * n_outs
    fn = jax.jit(
        shard_map(
            _body, mesh=mesh, in_specs=in_specs, out_specs=out_specs,
            check_rep=False,
        ),
        donate_argnums=donate,
        keep_unused=True,
    )
    sharding = NamedSharding(mesh, PartitionSpec("core"))
    return {
        "fn": fn,
        "in_names": in_names,
        "out_names": out_names,
        "out_avals": out_avals,
        "zero_out_specs": zero_out_specs,
        "sharding": sharding,
    }


def _get_runner(reps=1):
    key = f"runner{reps}"
    if key not in _CACHE:
        _CACHE[key] = _make_runner(_get_nc(reps))
    return _CACHE[key]


def _exec_spmd(in_maps, reps=1):
    """Run the cached executor on per-core input dicts; returns per-core
    output dicts."""
    import jax

    r = _get_runner(reps)
    concat_in = [
        np.concatenate([np.asarray(m[name]) for m in in_maps], axis=0)
        for name in r["in_names"]
    ]
    zeros = [
        np.zeros((E * shape[0], *shape[1:]), dtype)
        for shape, dtype in r["zero_out_specs"]
    ]
    out_arrs = r["fn"](*concat_in, *zeros)
    results = []
    for c in range(E):
        results.append(
            {
                name: np.asarray(out_arrs[i]).reshape(
                    E, *r["out_avals"][i].shape
                )[c]
                for i, name in enumerate(r["out_names"])
            }
        )
    return results


def _prepare_in_maps(x, w1, b1, w2, b2):
    bf16 = ml_dtypes.bfloat16
    in_maps = []
    for e in range(E):
        x_e = np.ascontiguousarray(x[:, e].reshape(NTOK, D).T).astype(bf16)
        w1_e = np.ascontiguousarray(w1[e]).astype(bf16)
        w2_e = np.ascontiguousarray(w2[e]).astype(bf16)
        b1_e = np.ascontiguousarray(
            b1[e].astype(np.float32).reshape(KO2, P).T
        )
        b2_e = np.ascontiguousarray(
            np.broadcast_to(b2[e].astype(np.float32), (P, D))
        )
        in_maps.append(
            {"xT": x_e, "w1": w1_e, "b1": b1_e, "w2": w2_e, "b2": b2_e}
        )
    return in_maps


def _run(x, w1, b1, w2, b2):
    in_maps = _prepare_in_maps(x, w1, b1, w2, b2)
    results = _exec_spmd(in_maps)
    out = np.empty((B, E, N, D), dtype=np.float32)
    for e in range(E):
        out[:, e] = results[e]["out"].reshape(B, N, D)
    return out


def kernel(x, w1, b1, w2, b2):
    return _run(x, w1, b1, w2, b2)


# revision 17
# speedup vs baseline: 121.1774x; 1.0062x over previous
"""Expert-parallel MoE FFN kernel for 8 Trainium2 NeuronCores.

Problem: x (B=4, E=8, N=1024, D=1024) f32; per-expert 2-layer GELU FFN
  h = gelu(x[:,e] @ w1[e] + b1[e]);  out[:,e] = h @ w2[e] + b2[e]
with w1 (E, D, H=4096), w2 (E, H, D).

Sharding: expert-parallel, one expert per core (E == n_cores == 8). Each
core's work is fully independent — no collectives.

Per-core device kernel (fused, weights resident in SBUF, bf16 matmuls with
fp32 PSUM accumulation):
  - host sends xT_e = x_e.T (D, NTOK) bf16 so no on-device transposes needed
  - mm1: hT (H, TN-chunk) = w1.T @ xT   [lhsT = w1 block, rhs = xT chunk]
  - gelu(tanh approx) + b1 on ScalarE during PSUM->SBUF eviction (bf16)
  - mm2: out (TN-chunk, D) = hT.T @ w2  [lhsT = hT block, rhs = w2 block]
  - + b2 on VectorE during PSUM->SBUF eviction (f32), DMA to DRAM
"""

import numpy as np
import ml_dtypes

B, E, N, D, H = 4, 8, 1024, 1024, 4096
NTOK = B * N            # 4096 tokens per expert
P = 128
TN = 256                # token chunk = mm1 moving free dim
NCH = NTOK // TN        # 16 chunks
KO1 = D // P            # 8 k-subtiles for mm1 (contract over D)
KO2 = H // P            # 32 k-subtiles for mm2 (contract over H) = mm1 m-tiles
MT = TN // P            # 2 token subtiles per chunk
NF2 = 512               # mm2 moving free dim (over D)
ND = D // NF2           # 2

_CACHE: dict = {}


def _build_nc(reps=1):
    """Build the per-core Bass program. reps>1 repeats the (idempotent)
    kernel body for marginal-time benchmarking."""
    import concourse.mybir as mybir
    import concourse.tile as tile
    from concourse import bacc

    bf16 = mybir.dt.bfloat16
    f32 = mybir.dt.float32
    gelu = mybir.ActivationFunctionType.Gelu_apprx_tanh
    alu_add = mybir.AluOpType.add

    nc = bacc.Bacc(None, target_bir_lowering=False, debug=False)

    xT = nc.dram_tensor("xT", [D, NTOK], bf16, kind="ExternalInput")
    w1 = nc.dram_tensor("w1", [D, H], bf16, kind="ExternalInput")
    b1 = nc.dram_tensor("b1", [P, KO2], f32, kind="ExternalInput")
    w2 = nc.dram_tensor("w2", [H, D], bf16, kind="ExternalInput")
    b2 = nc.dram_tensor("b2", [P, D], f32, kind="ExternalInput")
    out = nc.dram_tensor("out", [NTOK, D], f32, kind="ExternalOutput")

    xT_v = xT.rearrange("(ko pi) n -> pi ko n", pi=P)     # (128, 8, 4096)
    w1_v = w1.rearrange("(ko pi) h -> pi ko h", pi=P)     # (128, 8, 4096)
    w2_v = w2.rearrange("(ko pi) d -> pi ko d", pi=P)     # (128, 32, 1024)
    out_v = out.rearrange("(mt pi) d -> pi mt d", pi=P)   # (128, 32, 1024)

    with tile.TileContext(nc) as tc:
        with (
            tc.tile_pool(name="wpool", bufs=1) as wpool,
            tc.tile_pool(name="xpool", bufs=2) as xpool,
            tc.tile_pool(name="hpool", bufs=6) as hpool,
            tc.tile_pool(name="opool", bufs=2) as opool,
            tc.tile_pool(name="phpool", bufs=2, space="PSUM") as phpool,
            tc.tile_pool(name="popool", bufs=1, space="PSUM") as popool,
            tc.tile_pool(name="popool2", bufs=2, space="PSUM") as popool2,
        ):
            # w1 split [ko][mg]: per-ko tiles chopped into H-column groups so
            # chunk 0's first matmuls only wait for the first ~1MB, not all
            # 8.4MB of w1. w2 split per-ko (mm2(m) waits only on piece m).
            MG = 4           # m-groups for w1 (H columns per group = H/MG)
            HG = H // MG
            w1_sb = [
                [wpool.tile([P, HG], bf16, name=f"w1_sb{ko}_{mg}") for mg in range(MG)]
                for ko in range(KO1)
            ]
            w2_sb = [wpool.tile([P, D], bf16, name=f"w2_sb{ko}") for ko in range(KO2)]
            b1_sb = wpool.tile([P, KO2], f32, name="b1_sb")
            b2_sb = wpool.tile([P, D], f32, name="b2_sb")

            def load_x(rep, t):
                xs = [
                    xpool.tile([P, TN], bf16, tag=f"x{ko}", name=f"x_{rep}_{t}_{ko}")
                    for ko in range(KO1)
                ]
                for ko in range(KO1):
                    nc.sync.dma_start(
                        out=xs[ko][:], in_=xT_v[:, ko, t * TN : (t + 1) * TN]
                    )
                return xs

            # DMA issue order = consumption order: x chunk 0, b1, then w1
            # m-group by m-group, with w2 pieces interleaved after the first
            # w1 group (mm2(m) starts ~2 m-steps after mm1(m)).
            x_next = load_x(0, 0)
            nc.sync.dma_start(out=b1_sb[:], in_=b1[:])
            for ko in range(KO1):
                nc.sync.dma_start(
                    out=w1_sb[ko][0][:], in_=w1_v[:, ko, 0:HG]
                )
            nc.sync.dma_start(out=b2_sb[:], in_=b2[:])
            for mg in range(1, MG):
                for ko in range(KO1):
                    nc.sync.dma_start(
                        out=w1_sb[ko][mg][:], in_=w1_v[:, ko, mg * HG : (mg + 1) * HG]
                    )
                # interleave a share of w2 pieces after each w1 group
                for ko in range((mg - 1) * KO2 // (MG - 1), mg * KO2 // (MG - 1)):
                    nc.sync.dma_start(out=w2_sb[ko][:], in_=w2_v[:, ko, :])

            # software pipeline: mm2 lags mm1 by LAG m-steps (across chunk
            # boundaries too) so PE never waits on the ScalarE gelu evict or
            # the previous chunk's PSUM eviction.
            LAG = 2
            pend_q = []  # entries: (h_sb, po, m, rep, t)

            def emit_mm2(h_sb, po, m, rep, t, final):
                for mt in range(MT):
                    for n in range(ND):
                        nc.tensor.matmul(
                            po[mt][:, n * NF2 : (n + 1) * NF2],
                            h_sb[:, mt * P : (mt + 1) * P],
                            w2_sb[m][:, n * NF2 : (n + 1) * NF2],
                            start=(m == 0),
                            stop=(m == KO2 - 1),
                        )
                    if final:
                        # evict this mt's accumulator right away (frees its
                        # PSUM slot before the next chunk's mm2 needs it)
                        o_sb = opool.tile(
                            [P, D], f32, tag=f"o{mt}", name=f"o{mt}_{rep}_{t}"
                        )
                        nc.vector.tensor_tensor(
                            o_sb[:], po[mt][:], b2_sb[:], alu_add
                        )
                        nc.sync.dma_start(
                            out=out_v[:, t * MT + mt, :], in_=o_sb[:]
                        )

            def pump(force=False):
                while pend_q and (force or len(pend_q) > LAG):
                    h_sb, po, m, rep, t = pend_q.pop(0)
                    emit_mm2(h_sb, po, m, rep, t, final=(m == KO2 - 1))

            for rep in range(reps):
              for t in range(NCH):
                x_sb = x_next
                po = [
                    popool.tile([P, D], f32, tag="po0", name=f"po0_{rep}_{t}"),
                    popool2.tile([P, D], f32, tag="po1", name=f"po1_{rep}_{t}"),
                ]
                for m in range(KO2):
                    mg, mo = divmod(m, KO2 // MG)
                    ph = phpool.tile([P, TN], f32, tag="ph", name=f"ph_{rep}_{t}_{m}")
                    for ko in range(KO1):
                        nc.tensor.matmul(
                            ph[:],
                            w1_sb[ko][mg][:, mo * P : (mo + 1) * P],
                            x_sb[ko][:],
                            start=(ko == 0),
                            stop=(ko == KO1 - 1),
                        )
                    h_sb = hpool.tile([P, TN], bf16, tag="h", name=f"h_{rep}_{t}_{m}")
                    nc.scalar.activation(
                        h_sb[:], ph[:], gelu, bias=b1_sb[:, m : m + 1], scale=1.0
                    )
                    pend_q.append((h_sb, po, m, rep, t))
                    pump()
                    if m == 0 and not (t == NCH - 1 and rep == reps - 1):
                        # prefetch next chunk's x while this chunk computes
                        tn, rn = (t + 1, rep) if t < NCH - 1 else (0, rep + 1)
                        x_next = load_x(rn, tn)
            pump(force=True)

    nc.compile()
    return nc


def _get_nc(reps=1):
    key = f"nc{reps}"
    if key not in _CACHE:
        _CACHE[key] = _build_nc(reps)
    return _CACHE[key]


def _make_runner(nc):
    """Build a jitted SPMD executor for an arbitrary finalized Bass module.

    Mirrors concourse.bass2jax.run_bass_via_pjrt's multi-core branch, but
    returns a reusable jitted function (no re-trace/re-compile per call).
    """
    import jax
    from jax.experimental.shard_map import shard_map
    from jax.sharding import Mesh, NamedSharding, PartitionSpec

    import concourse.mybir as mybir
    from concourse import bass2jax

    bass2jax.install_neuronx_cc_hook()

    partition_name = (
        nc.partition_id_tensor.name if nc.partition_id_tensor else None
    )
    in_names = []
    out_names = []
    out_avals = []
    zero_out_specs = []
    for alloc in nc.m.functions[0].allocations:
        if not isinstance(alloc, mybir.MemoryLocationSet):
            continue
        name = alloc.memorylocations[0].name
        if alloc.kind == "ExternalInput":
            if name != partition_name:
                in_names.append(name)
        elif alloc.kind == "ExternalOutput":
            shape = tuple(alloc.tensor_shape)
            dtype = mybir.dt.np(alloc.dtype)
            out_names.append(name)
            out_avals.append(jax.core.ShapedArray(shape, dtype))
            zero_out_specs.append((shape, dtype))
    n_params = len(in_names)
    n_outs = len(out_names)
    all_in_names = list(in_names) + list(out_names)
    if partition_name is not None:
        all_in_names.append(partition_name)
    donate = tuple(range(n_params, n_params + n_outs))

    def _body(*args):
        operands = list(args)
        if partition_name is not None:
            operands.append(bass2jax.partition_id_tensor())
        outs = bass2jax._bass_exec_p.bind(
            *operands,
            out_avals=tuple(out_avals),
            in_names=tuple(all_in_names),
            out_names=tuple(out_names),
            lowering_input_output_aliases=(),
            sim_require_finite=True,
            sim_require_nnan=True,
            nc=nc,
        )
        return tuple(outs)

    devices = jax.devices()[:E]
    mesh = Mesh(np.asarray(devices), ("core",))
    in_specs = (PartitionSpec("core"),) * (n_params + n_outs)
    out_specs = (PartitionSpec("core"),) * n_outs
    fn = jax.jit(
        shard_map(
            _body, mesh=mesh, in_specs=in_specs, out_specs=out_specs,
            check_rep=False,
        ),
        donate_argnums=donate,
        keep_unused=True,
    )
    sharding = NamedSharding(mesh, PartitionSpec("core"))
    return {
        "fn": fn,
        "in_names": in_names,
        "out_names": out_names,
        "out_avals": out_avals,
        "zero_out_specs": zero_out_specs,
        "sharding": sharding,
    }


def _get_runner(reps=1):
    key = f"runner{reps}"
    if key not in _CACHE:
        _CACHE[key] = _make_runner(_get_nc(reps))
    return _CACHE[key]


def _exec_spmd(in_maps, reps=1):
    """Run the cached executor on per-core input dicts; returns per-core
    output dicts."""
    import jax

    r = _get_runner(reps)
    concat_in = [
        np.concatenate([np.asarray(m[name]) for m in in_maps], axis=0)
        for name in r["in_names"]
    ]

    def _call():
        zeros = [
            np.zeros((E * shape[0], *shape[1:]), dtype)
            for shape, dtype in r["zero_out_specs"]
        ]
        outs = r["fn"](*concat_in, *zeros)
        for o in outs:
            o.block_until_ready()
        return outs

    try:
        out_arrs = _call()
    except Exception:
        # transient device errors (e.g. NRT exec-unit unrecoverable) have
        # been observed to clear on retry
        import time as _time

        _time.sleep(5.0)
        out_arrs = _call()
    results = []
    for c in range(E):
        results.append(
            {
                name: np.asarray(out_arrs[i]).reshape(
                    E, *r["out_avals"][i].shape
                )[c]
                for i, name in enumerate(r["out_names"])
            }
        )
    return results


def _prepare_in_maps(x, w1, b1, w2, b2):
    bf16 = ml_dtypes.bfloat16
    in_maps = []
    for e in range(E):
        x_e = np.ascontiguousarray(x[:, e].reshape(NTOK, D).T).astype(bf16)
        w1_e = np.ascontiguousarray(w1[e]).astype(bf16)
        w2_e = np.ascontiguousarray(w2[e]).astype(bf16)
        b1_e = np.ascontiguousarray(
            b1[e].astype(np.float32).reshape(KO2, P).T
        )
        b2_e = np.ascontiguousarray(
            np.broadcast_to(b2[e].astype(np.float32), (P, D))
        )
        in_maps.append(
            {"xT": x_e, "w1": w1_e, "b1": b1_e, "w2": w2_e, "b2": b2_e}
        )
    return in_maps


def _run(x, w1, b1, w2, b2):
    in_maps = _prepare_in_maps(x, w1, b1, w2, b2)
    results = _exec_spmd(in_maps)
    out = np.empty((B, E, N, D), dtype=np.float32)
    for e in range(E):
        out[:, e] = results[e]["out"].reshape(B, N, D)
    return out


def kernel(x, w1, b1, w2, b2):
    return _run(x, w1, b1, w2, b2)
